# revision 2
# baseline (speedup 1.0000x reference)
"""TRN2 Bass kernel for nn_GraphVectorEncoder (3-layer TransformerConv GNN + mean pool).

Strategy: partition nodes (and incoming edges) across 8 cores by contiguous
dst ranges. Per core, nodes are degree-sorted into 128-row tiles with K edge
slots along the free axis (dense-bucket layout), so segment softmax = row-wise
ops. Algebraic folding removes k/v tables entirely:
    alpha = x_src . (Wk_h @ q_dst)/sqrt(hd) + c_dst
    agg   = (sum_j p_j x_src_j) @ Wv_h + (sum p) bv_h
Edge phase gathers only x_src rows (512B) via indirect DMA. Layer boundaries
all-gather h across cores. Mean-pool partials are returned per core and
combined on host.
"""

import numpy as np

N, E, G = 50000, 800000, 64
D = 128
NC = 8
NLOC = N // NC           # 6250
TILES = (NLOC + 127) // 128   # 49
PADLOC = TILES * 128     # 6272
NPAD = NC * PADLOC       # 50176
LAYER_HEADS = [2, 2, 1]
LAYER_HD = [64, 64, 128]

_CACHE = {}


def _build(meta):
    import concourse.bass as bass
    import concourse.mybir as mybir
    import concourse.tile as tile
    from concourse import bacc
    from concourse.masks import make_identity

    Kt = meta["Kt"]
    SLOTS = int(Kt.sum())
    off = np.concatenate([[0], np.cumsum(Kt)]).astype(int)

    nc = bacc.Bacc("TRN2", target_bir_lowering=False, debug=False, num_devices=NC)
    f32 = mybir.dt.float32
    i32 = mybir.dt.int32

    xin = nc.dram_tensor("xin", [N, D], f32, kind="ExternalInput")
    pnat = nc.dram_tensor("pnat", [128, TILES], i32, kind="ExternalInput")
    ppad = nc.dram_tensor("ppad", [128, TILES], i32, kind="ExternalInput")
    scat = nc.dram_tensor("scat", [128, TILES], i32, kind="ExternalInput")
    gnat = nc.dram_tensor("gnat", [128, SLOTS], i32, kind="ExternalInput")
    gpad = nc.dram_tensor("gpad", [128, SLOTS], i32, kind="ExternalInput")
    mbig = nc.dram_tensor("mbig", [128, SLOTS], f32, kind="ExternalInput")
    oneh = nc.dram_tensor("oneh", [128, TILES * G], f32, kind="ExternalInput")
    Fs = [2 * 128 + 2 + 128, 2 * 128 + 2 + 128, 128 + 1 + 128]
    wts, bts, wvs, bvs = [], [], [], []
    for li in range(3):
        wts.append(nc.dram_tensor(f"wt{li}", [128, Fs[li]], f32, kind="ExternalInput"))
        bts.append(nc.dram_tensor(f"bt{li}", [128, Fs[li]], f32, kind="ExternalInput"))
        wvs.append(nc.dram_tensor(f"wv{li}", [128, 128], f32, kind="ExternalInput"))
        bvs.append(nc.dram_tensor(f"bv{li}", [128, 128], f32, kind="ExternalInput"))
    pool_out = nc.dram_tensor("pool_out", [G, D], f32, kind="ExternalOutput")

    h_bounce = nc.dram_tensor("h_bounce", [PADLOC, D], f32)
    h_full = nc.dram_tensor("h_full", [NPAD, D], f32)

    with tile.TileContext(nc) as tc:
        with (
            tc.tile_pool(name="const", bufs=1) as cp,
            tc.tile_pool(name="work", bufs=2) as wp,
            tc.tile_pool(name="gw", bufs=2) as gp,
            tc.tile_pool(name="ps", bufs=1, space="PSUM") as pp,
            tc.tile_pool(name="pool_ps", bufs=1, space="PSUM") as ppool,
        ):
            # ---- constants ----
            ident = cp.tile([128, 128], f32)
            make_identity(nc, ident[:])
            pnat_sb = cp.tile([128, TILES], i32)
            nc.sync.dma_start(pnat_sb[:], pnat[:])
            ppad_sb = cp.tile([128, TILES], i32)
            nc.sync.dma_start(ppad_sb[:], ppad[:])
            scat_sb = cp.tile([128, TILES], i32)
            nc.sync.dma_start(scat_sb[:], scat[:])
            gnat_sb = cp.tile([128, SLOTS], i32)
            nc.sync.dma_start(gnat_sb[:], gnat[:])
            gpad_sb = cp.tile([128, SLOTS], i32)
            nc.sync.dma_start(gpad_sb[:], gpad[:])
            mbig_sb = cp.tile([128, SLOTS], f32)
            nc.sync.dma_start(mbig_sb[:], mbig[:])
            oneh_sb = cp.tile([128, TILES * G], f32)
            nc.sync.dma_start(oneh_sb[:], oneh[:])
            wt_sb, bt_sb, wv_sb, bv_sb = [], [], [], []
            for li in range(3):
                w = cp.tile([128, Fs[li]], f32)
                nc.sync.dma_start(w[:], wts[li][:])
                wt_sb.append(w)
                b = cp.tile([128, Fs[li]], f32)
                nc.sync.dma_start(b[:], bts[li][:])
                bt_sb.append(b)
                wv = cp.tile([128, 128], f32)
                nc.sync.dma_start(wv[:], wvs[li][:])
                wv_sb.append(wv)
                bv = cp.tile([128, 128], f32)
                nc.sync.dma_start(bv[:], bvs[li][:])
                bv_sb.append(bv)

            pool_ps = ppool.tile([G, D], f32, space="PSUM")

            for li in range(3):
                H = LAYER_HEADS[li]
                hd = LAYER_HD[li]
                F = Fs[li]
                src_tbl = xin if li == 0 else h_full
                perm_sb = pnat_sb if li == 0 else ppad_sb
                gidx_sb = gnat_sb if li == 0 else gpad_sb

                for t in range(TILES):
                    K = int(Kt[t])
                    o0 = int(off[t])
                    # node features for this tile
                    xg = wp.tile([128, D], f32)
                    nc.gpsimd.indirect_dma_start(
                        out=xg[:], out_offset=None, in_=src_tbl[:],
                        in_offset=bass.IndirectOffsetOnAxis(
                            ap=perm_sb[:, t:t + 1], axis=0))
                    xgT_ps = pp.tile([128, D], f32, space="PSUM")
                    nc.tensor.transpose(out=xgT_ps[:], in_=xg[:], identity=ident[:])
                    xT = wp.tile([128, D], f32)
                    nc.vector.tensor_copy(out=xT[:], in_=xgT_ps[:])
                    p1 = pp.tile([128, F], f32, space="PSUM")
                    nc.tensor.matmul(p1[:], lhsT=xT[:], rhs=wt_sb[li][:],
                                     start=True, stop=True)
                    qts = wp.tile([128, F], f32)
                    nc.vector.tensor_tensor(out=qts[:], in0=p1[:], in1=bt_sb[li][:],
                                            op=mybir.AluOpType.add)

                    if K > 0:
                        Gt = gp.tile([128, K * D], f32)
                        for j in range(K):
                            nc.gpsimd.indirect_dma_start(
                                out=Gt[:, j * D:(j + 1) * D], out_offset=None,
                                in_=src_tbl[:],
                                in_offset=bass.IndirectOffsetOnAxis(
                                    ap=gidx_sb[:, o0 + j:o0 + j + 1], axis=0))
                        w_sc = gp.tile([128, K * D], f32)
                        alpha = wp.tile([128, H * K], f32)
                        pexp = wp.tile([128, H * K], f32)
                        recs = wp.tile([128, 2 * H], f32)  # [denraw | rec] per head
                        for h in range(H):
                            qb = bass.AP(qts.tensor, qts[:].offset + h * D,
                                         [qts[:].ap[0], [0, K], [1, D]])
                            nc.vector.tensor_tensor(
                                out=w_sc[:].rearrange("p (k d) -> p k d", k=K),
                                in0=Gt[:].rearrange("p (k d) -> p k d", k=K),
                                in1=qb, op=mybir.AluOpType.mult)
                            nc.vector.tensor_reduce(
                                out=alpha[:, h * K:(h + 1) * K],
                                in_=w_sc[:].rearrange("p (k d) -> p k d", k=K),
                                axis=mybir.AxisListType.X, op=mybir.AluOpType.add)
                            # + c term (per-partition scalar)
                            nc.vector.tensor_scalar(
                                out=alpha[:, h * K:(h + 1) * K],
                                in0=alpha[:, h * K:(h + 1) * K],
                                scalar1=qts[:, H * D + h:H * D + h + 1],
                                scalar2=None, op0=mybir.AluOpType.add)
                            # + (-1e30) on padded slots
                            nc.vector.tensor_tensor(
                                out=alpha[:, h * K:(h + 1) * K],
                                in0=alpha[:, h * K:(h + 1) * K],
                                in1=mbig_sb[:, o0:o0 + K],
                                op=mybir.AluOpType.add)
                            amax = wp.tile([128, 1], f32)
                            nc.vector.tensor_reduce(
                                out=amax[:], in_=alpha[:, h * K:(h + 1) * K],
                                axis=mybir.AxisListType.X, op=mybir.AluOpType.max)
                            nc.vector.tensor_scalar(
                                out=alpha[:, h * K:(h + 1) * K],
                                in0=alpha[:, h * K:(h + 1) * K],
                                scalar1=amax[:, :1], scalar2=None,
                                op0=mybir.AluOpType.subtract)
                            nc.scalar.activation(pexp[:, h * K:(h + 1) * K],
                                                 alpha[:, h * K:(h + 1) * K],
                                                 mybir.ActivationFunctionType.Exp)
                            nc.vector.tensor_reduce(
                                out=recs[:, h:h + 1], in_=pexp[:, h * K:(h + 1) * K],
                                axis=mybir.AxisListType.X, op=mybir.AluOpType.add)
                            den = wp.tile([128, 1], f32)
                            nc.vector.tensor_scalar(
                                out=den[:], in0=recs[:, h:h + 1], scalar1=1e-16,
                                scalar2=None, op0=mybir.AluOpType.add)
                            nc.vector.reciprocal(out=recs[:, H + h:H + h + 1],
                                                 in_=den[:])

                    ot = wp.tile([128, D], f32)  # output features (pre-relu)
                    for h in range(H):
                        if K > 0:
                            pre = wp.tile([128, D], f32)
                            tmp = wp.tile([128, D], f32)
                            nc.scalar.activation(
                                pre[:], Gt[:, :D],
                                mybir.ActivationFunctionType.Copy,
                                scale=pexp[:, h * K:h * K + 1])
                            for j in range(1, K):
                                dst_t = tmp if (j % 2) else pre
                                nc.scalar.activation(
                                    tmp[:], Gt[:, j * D:(j + 1) * D],
                                    mybir.ActivationFunctionType.Copy,
                                    scale=pexp[:, h * K + j:h * K + j + 1])
                                nc.vector.tensor_tensor(
                                    out=pre[:], in0=pre[:], in1=tmp[:],
                                    op=mybir.AluOpType.add)
                            preT_ps = pp.tile([128, D], f32, space="PSUM")
                            nc.tensor.transpose(out=preT_ps[:], in_=pre[:],
                                                identity=ident[:])
                            preT = wp.tile([128, D], f32)
                            nc.vector.tensor_copy(out=preT[:], in_=preT_ps[:])
                            agg_ps = pp.tile([128, hd], f32, space="PSUM")
                            nc.tensor.matmul(agg_ps[:], lhsT=preT[:],
                                             rhs=wv_sb[li][:, h * hd:(h + 1) * hd],
                                             start=True, stop=True)
                            # + denraw * bv_h, then * rec
                            bvt = wp.tile([128, hd], f32)
                            nc.vector.tensor_scalar(
                                out=bvt[:], in0=bv_sb[li][:, h * hd:(h + 1) * hd],
                                scalar1=recs[:, h:h + 1], scalar2=None,
                                op0=mybir.AluOpType.mult)
                            nc.vector.tensor_tensor(
                                out=bvt[:], in0=bvt[:], in1=agg_ps[:],
                                op=mybir.AluOpType.add)
                            nc.vector.tensor_scalar(
                                out=ot[:, h * hd:(h + 1) * hd], in0=bvt[:],
                                scalar1=recs[:, H + h:H + h + 1], scalar2=None,
                                op0=mybir.AluOpType.mult)
                        else:
                            nc.vector.memset(ot[:, h * hd:(h + 1) * hd], 0.0)
                    # + skip, relu
                    nc.vector.tensor_tensor(out=ot[:], in0=ot[:],
                                            in1=qts[:, H * D + H:H * D + H + D],
                                            op=mybir.AluOpType.add)
                    ht = wp.tile([128, D], f32)
                    nc.scalar.activation(ht[:], ot[:],
                                         mybir.ActivationFunctionType.Relu)
                    if li < 2:
                        nc.gpsimd.indirect_dma_start(
                            out=h_bounce[:],
                            out_offset=bass.IndirectOffsetOnAxis(
                                ap=scat_sb[:, t:t + 1], axis=0),
                            in_=ht[:], in_offset=None)
                    else:
                        nc.tensor.matmul(
                            pool_ps[:], lhsT=oneh_sb[:, t * G:(t + 1) * G],
                            rhs=ht[:], start=(t == 0), stop=(t == TILES - 1))
                if li < 2:
                    nc.gpsimd.collective_compute(
                        "AllGather", mybir.AluOpType.bypass,
                        replica_groups=[list(range(NC))],
                        ins=[h_bounce.ap().opt()],
                        outs=[h_full.ap().opt()])
            pout_sb = cp.tile([G, D], f32)
            nc.vector.tensor_copy(out=pout_sb[:], in_=pool_ps[:])
            nc.sync.dma_start(pool_out[:], pout_sb[:])
    nc.compile()
    return nc


def _prep(x, edge_index, batch, weights):
    src = np.asarray(edge_index[0], dtype=np.int64)
    dst = np.asarray(edge_index[1], dtype=np.int64)
    batch = np.asarray(batch, dtype=np.int64)
    deg = np.bincount(dst, minlength=N)

    # incoming edge lists grouped by dst
    order = np.argsort(dst, kind="stable")
    src_sorted = src[order]
    starts = np.concatenate([[0], np.cumsum(deg)]).astype(np.int64)

    perm = np.zeros((NC, PADLOC), dtype=np.int64)
    degs = np.zeros((NC, PADLOC), dtype=np.int64)
    valid = np.zeros((NC, PADLOC), dtype=bool)
    for c in range(NC):
        ids = np.arange(c * NLOC, (c + 1) * NLOC)
        d = deg[ids]
        o = np.argsort(-d, kind="stable")
        perm[c, :NLOC] = ids[o]
        degs[c, :NLOC] = d[o]
        valid[c, :NLOC] = True

    Kt = degs.reshape(NC, TILES, 128).max(axis=(0, 2)).astype(np.int64)
    SLOTS = int(Kt.sum())
    off = np.concatenate([[0], np.cumsum(Kt)]).astype(int)

    def pad_map(ids):
        return (ids // NLOC) * PADLOC + (ids % NLOC)

    ins = []
    for c in range(NC):
        gnat = np.zeros((128, SLOTS), dtype=np.int32)
        mb = np.full((128, SLOTS), -1e30, dtype=np.float32)
        for t in range(TILES):
            K = int(Kt[t])
            for p in range(128):
                i = t * 128 + p
                if not valid[c, i]:
                    continue
                n_id = perm[c, i]
                d = int(deg[n_id])
                if d > 0:
                    ss = src_sorted[starts[n_id]:starts[n_id + 1]]
                    gnat[p, off[t]:off[t] + d] = ss
                    mb[p, off[t]:off[t] + d] = 0.0
        gpad = (pad_map(gnat.astype(np.int64))).astype(np.int32)
        pn = np.zeros((128, TILES), dtype=np.int32)
        ppd = np.zeros((128, TILES), dtype=np.int32)
        sc = np.zeros((128, TILES), dtype=np.int32)
        ohb = np.zeros((128, TILES * G), dtype=np.float32)
        ndum = 0
        for t in range(TILES):
            for p in range(128):
                i = t * 128 + p
                if valid[c, i]:
                    n_id = perm[c, i]
                    pn[p, t] = n_id
                    ppd[p, t] = pad_map(np.int64(n_id))
                    sc[p, t] = n_id - c * NLOC
                    ohb[p, t * G + int(batch[n_id])] = 1.0
                else:
                    pn[p, t] = 0
                    ppd[p, t] = 0
                    sc[p, t] = NLOC + ndum
                    ndum += 1
        m = dict(xin=np.ascontiguousarray(x, dtype=np.float32),
                 pnat=pn, ppad=ppd, scat=sc, gnat=gnat, gpad=gpad,
                 mbig=mb, oneh=ohb)
        ins.append(m)

    # folded weights
    for li in range(3):
        H = LAYER_HEADS[li]
        hd = LAYER_HD[li]
        Wq, bq, Wk, bk, Wv, bv, Ws, bs = weights[li]
        din = Wq.shape[0]
        s = 1.0 / np.sqrt(hd)
        Wq_h = Wq.reshape(din, H, hd)
        Wk_h = Wk.reshape(din, H, hd)
        bq_h = bq.reshape(H, hd)
        bk_h = bk.reshape(H, hd)
        wt = np.zeros((128, H * 128 + H + 128), dtype=np.float32)
        bt = np.zeros((128, H * 128 + H + 128), dtype=np.float32)
        for h in range(H):
            wt[:, h * 128:(h + 1) * 128] = (Wq_h[:, h] @ Wk_h[:, h].T) * s
            bt[:, h * 128:(h + 1) * 128] = ((bq_h[h] @ Wk_h[:, h].T) * s)[None, :]
            wt[:, H * 128 + h] = (Wq_h[:, h] @ bk_h[h]) * s
            bt[:, H * 128 + h] = float(bq_h[h] @ bk_h[h]) * s
        wt[:, H * 128 + H:] = Ws
        bt[:, H * 128 + H:] = bs[None, :]
        wv = np.zeros((128, 128), dtype=np.float32)
        bvb = np.zeros((128, 128), dtype=np.float32)
        Wv_h = Wv.reshape(din, H, hd)
        bv_h = bv.reshape(H, hd)
        for h in range(H):
            wv[:, h * hd:(h + 1) * hd] = Wv_h[:, h]
            bvb[:, h * hd:(h + 1) * hd] = bv_h[h][None, :]
        for m in ins:
            m[f"wt{li}"] = wt
            m[f"bt{li}"] = bt
            m[f"wv{li}"] = wv
            m[f"bv{li}"] = bvb
    return ins, Kt, batch


def kernel(**inputs):
    x = np.asarray(inputs["x"], dtype=np.float32)
    weights = []
    for li in range(1, 4):
        weights.append(tuple(np.asarray(inputs[f"{nm}{li}"], dtype=np.float32)
                             for nm in ("Wq", "bq", "Wk", "bk", "Wv", "bv", "Ws", "bs")))
    ins, Kt, batch = _prep(x, inputs["edge_index"], inputs["batch"], weights)

    key = tuple(Kt.tolist())
    if key not in _CACHE:
        _CACHE[key] = _build({"Kt": Kt})
    nc = _CACHE[key]

    from concourse.bass_utils import run_bass_kernel_spmd
    r = run_bass_kernel_spmd(nc, ins, core_ids=list(range(NC)))
    parts = np.stack([r.results[c]["pool_out"] for c in range(NC)])  # [NC,G,D]
    sums = parts.sum(axis=0)
    cnts = np.bincount(np.asarray(batch, dtype=np.int64), minlength=G).astype(np.float32)
    return (sums / np.maximum(cnts, 1.0)[:, None]).astype(np.float32)


# revision 6
# speedup vs baseline: 7.5261x; 7.5261x over previous
"""TRN2 Bass kernel for nn_GraphVectorEncoder (3-layer TransformerConv GNN + mean pool).

Strategy: partition nodes (and incoming edges) across 8 cores by contiguous
dst ranges. Per core, nodes are degree-sorted into 128-row tiles with K edge
slots along the free axis (dense-bucket layout), so segment softmax = row-wise
ops. Algebraic folding removes k/v tables entirely:
    alpha = x_src . (Wk_h @ q_dst)/sqrt(hd) + c_dst
    agg   = (sum_j p_j x_src_j) @ Wv_h + (sum p) bv_h
Edge phase gathers only x_src rows (512B) via indirect DMA. Layer boundaries
all-gather h across cores. Mean-pool partials are returned per core and
combined on host.
"""

import numpy as np

N, E, G = 50000, 800000, 64
D = 128
NC = 8
NLOC = N // NC           # 6250
TILES = (NLOC + 127) // 128   # 49
PADLOC = TILES * 128     # 6272
NPAD = NC * PADLOC       # 50176
LAYER_HEADS = [2, 2, 1]
LAYER_HD = [64, 64, 128]

_CACHE = {}


def _build(meta):
    import concourse.bass as bass
    import concourse.mybir as mybir
    import concourse.tile as tile
    from concourse import bacc
    from concourse.masks import make_identity

    Kt = meta["Kt"]
    SLOTS = int(Kt.sum())
    off = np.concatenate([[0], np.cumsum(Kt)]).astype(int)

    nc = bacc.Bacc("TRN2", target_bir_lowering=False, debug=False, num_devices=NC)
    f32 = mybir.dt.float32
    i32 = mybir.dt.int32

    xin = nc.dram_tensor("xin", [N, D], f32, kind="ExternalInput")
    pnat = nc.dram_tensor("pnat", [128, TILES], i32, kind="ExternalInput")
    ppad = nc.dram_tensor("ppad", [128, TILES], i32, kind="ExternalInput")
    scat = nc.dram_tensor("scat", [128, TILES], i32, kind="ExternalInput")
    gnat = nc.dram_tensor("gnat", [128, SLOTS], i32, kind="ExternalInput")
    gpad = nc.dram_tensor("gpad", [128, SLOTS], i32, kind="ExternalInput")
    mbig = nc.dram_tensor("mbig", [128, SLOTS], f32, kind="ExternalInput")
    oneh = nc.dram_tensor("oneh", [128, TILES * G], f32, kind="ExternalInput")
    Fs = [2 * 128 + 2 + 128, 2 * 128 + 2 + 128, 128 + 1 + 128]
    wts, bts, wvs, bvs = [], [], [], []
    for li in range(3):
        wts.append(nc.dram_tensor(f"wt{li}", [128, Fs[li]], f32, kind="ExternalInput"))
        bts.append(nc.dram_tensor(f"bt{li}", [128, Fs[li]], f32, kind="ExternalInput"))
        wvs.append(nc.dram_tensor(f"wv{li}", [128, 128], f32, kind="ExternalInput"))
        bvs.append(nc.dram_tensor(f"bv{li}", [128, 128], f32, kind="ExternalInput"))
    pool_out = nc.dram_tensor("pool_out", [G, D], f32, kind="ExternalOutput")

    h_bounce = nc.dram_tensor("h_bounce", [PADLOC, D], f32)
    h_full = nc.dram_tensor("h_full", [NPAD, D], f32)

    with tile.TileContext(nc) as tc:
        with (
            tc.tile_pool(name="const", bufs=1) as cp,
            tc.tile_pool(name="work", bufs=2) as wp,
            tc.tile_pool(name="gw", bufs=2) as gp,
            tc.tile_pool(name="ps", bufs=2, space="PSUM") as pp,
            tc.tile_pool(name="ps_agg", bufs=1, space="PSUM") as pagg,
            tc.tile_pool(name="pool_ps", bufs=1, space="PSUM") as ppool,
        ):
            # ---- constants ----
            ident = cp.tile([128, 128], f32)
            make_identity(nc, ident[:])
            pnat_sb = cp.tile([128, TILES], i32)
            nc.sync.dma_start(pnat_sb[:], pnat[:])
            ppad_sb = cp.tile([128, TILES], i32)
            nc.sync.dma_start(ppad_sb[:], ppad[:])
            scat_sb = cp.tile([128, TILES], i32)
            nc.sync.dma_start(scat_sb[:], scat[:])
            gnat_sb = cp.tile([128, SLOTS], i32)
            nc.sync.dma_start(gnat_sb[:], gnat[:])
            gpad_sb = cp.tile([128, SLOTS], i32)
            nc.sync.dma_start(gpad_sb[:], gpad[:])
            mbig_sb = cp.tile([128, SLOTS], f32)
            nc.sync.dma_start(mbig_sb[:], mbig[:])
            oneh_sb = cp.tile([128, TILES * G], f32)
            nc.sync.dma_start(oneh_sb[:], oneh[:])
            wt_sb, bt_sb, wv_sb, bv_sb = [], [], [], []
            for li in range(3):
                w = cp.tile([128, Fs[li]], f32)
                nc.sync.dma_start(w[:], wts[li][:])
                wt_sb.append(w)
                b = cp.tile([128, Fs[li]], f32)
                nc.sync.dma_start(b[:], bts[li][:])
                bt_sb.append(b)
                wv = cp.tile([128, 128], f32)
                nc.sync.dma_start(wv[:], wvs[li][:])
                wv_sb.append(wv)
                bv = cp.tile([128, 128], f32)
                nc.sync.dma_start(bv[:], bvs[li][:])
                bv_sb.append(bv)

            pool_ps = ppool.tile([G, D], f32, space="PSUM")

            for li in range(3):
                H = LAYER_HEADS[li]
                hd = LAYER_HD[li]
                F = Fs[li]
                src_tbl = xin if li == 0 else h_full
                perm_sb = pnat_sb if li == 0 else ppad_sb
                gidx_sb = gnat_sb if li == 0 else gpad_sb

                for t in range(TILES):
                    K = int(Kt[t])
                    o0 = int(off[t])
                    # node features for this tile
                    xg = wp.tile([128, D], f32)
                    nc.gpsimd.indirect_dma_start(
                        out=xg[:], out_offset=None, in_=src_tbl[:],
                        in_offset=bass.IndirectOffsetOnAxis(
                            ap=perm_sb[:, t:t + 1], axis=0))
                    xgT_ps = pp.tile([128, D], f32, space="PSUM")
                    nc.tensor.transpose(out=xgT_ps[:], in_=xg[:], identity=ident[:])
                    xT = wp.tile([128, D], f32)
                    nc.vector.tensor_copy(out=xT[:], in_=xgT_ps[:])
                    p1 = pp.tile([128, F], f32, space="PSUM")
                    nc.tensor.matmul(p1[:], lhsT=xT[:], rhs=wt_sb[li][:],
                                     start=True, stop=True)
                    qts = wp.tile([128, F], f32)
                    nc.vector.tensor_tensor(out=qts[:], in0=p1[:], in1=bt_sb[li][:],
                                            op=mybir.AluOpType.add)

                    if K > 0:
                        Gt = gp.tile([128, K * D], f32)
                        for j in range(K):
                            nc.gpsimd.indirect_dma_start(
                                out=Gt[:, j * D:(j + 1) * D], out_offset=None,
                                in_=src_tbl[:],
                                in_offset=bass.IndirectOffsetOnAxis(
                                    ap=gidx_sb[:, o0 + j:o0 + j + 1], axis=0))
                        w_sc = gp.tile([128, K * D], f32)
                        alpha = wp.tile([128, H * K], f32)
                        pexp = wp.tile([128, H * K], f32)
                        recs = wp.tile([128, 2 * H], f32)  # [denraw | rec] per head
                        for h in range(H):
                            qb = bass.AP(qts.tensor, qts[:].offset + h * D,
                                         [qts[:].ap[0], [0, K], [1, D]])
                            nc.vector.tensor_tensor(
                                out=w_sc[:].rearrange("p (k d) -> p k d", k=K),
                                in0=Gt[:].rearrange("p (k d) -> p k d", k=K),
                                in1=qb, op=mybir.AluOpType.mult)
                            nc.vector.tensor_reduce(
                                out=alpha[:, h * K:(h + 1) * K],
                                in_=w_sc[:].rearrange("p (k d) -> p k d", k=K),
                                axis=mybir.AxisListType.X, op=mybir.AluOpType.add)
                            # + c term (per-partition scalar)
                            nc.vector.tensor_scalar(
                                out=alpha[:, h * K:(h + 1) * K],
                                in0=alpha[:, h * K:(h + 1) * K],
                                scalar1=qts[:, H * D + h:H * D + h + 1],
                                scalar2=None, op0=mybir.AluOpType.add)
                            # + (-1e30) on padded slots
                            nc.vector.tensor_tensor(
                                out=alpha[:, h * K:(h + 1) * K],
                                in0=alpha[:, h * K:(h + 1) * K],
                                in1=mbig_sb[:, o0:o0 + K],
                                op=mybir.AluOpType.add)
                            amax = wp.tile([128, 1], f32)
                            nc.vector.tensor_reduce(
                                out=amax[:], in_=alpha[:, h * K:(h + 1) * K],
                                axis=mybir.AxisListType.X, op=mybir.AluOpType.max)
                            nc.vector.tensor_scalar(
                                out=alpha[:, h * K:(h + 1) * K],
                                in0=alpha[:, h * K:(h + 1) * K],
                                scalar1=amax[:, :1], scalar2=None,
                                op0=mybir.AluOpType.subtract)
                            nc.scalar.activation(pexp[:, h * K:(h + 1) * K],
                                                 alpha[:, h * K:(h + 1) * K],
                                                 mybir.ActivationFunctionType.Exp)
                            nc.vector.tensor_reduce(
                                out=recs[:, h:h + 1], in_=pexp[:, h * K:(h + 1) * K],
                                axis=mybir.AxisListType.X, op=mybir.AluOpType.add)
                            den = wp.tile([128, 1], f32)
                            nc.vector.tensor_scalar(
                                out=den[:], in0=recs[:, h:h + 1], scalar1=1e-16,
                                scalar2=None, op0=mybir.AluOpType.add)
                            nc.vector.reciprocal(out=recs[:, H + h:H + h + 1],
                                                 in_=den[:])

                    ot = wp.tile([128, D], f32)  # output features (pre-relu)
                    for h in range(H):
                        if K > 0:
                            pre = wp.tile([128, D], f32)
                            tmp_a = wp.tile([128, D], f32)
                            tmp_b = wp.tile([128, D], f32)
                            tmps = [tmp_a, tmp_b]
                            nc.scalar.activation(
                                pre[:], Gt[:, :D],
                                mybir.ActivationFunctionType.Copy,
                                scale=pexp[:, h * K:h * K + 1])
                            for j in range(1, K):
                                tm = tmps[j % 2]
                                nc.scalar.activation(
                                    tm[:], Gt[:, j * D:(j + 1) * D],
                                    mybir.ActivationFunctionType.Copy,
                                    scale=pexp[:, h * K + j:h * K + j + 1])
                                nc.vector.tensor_tensor(
                                    out=pre[:], in0=pre[:], in1=tm[:],
                                    op=mybir.AluOpType.add)
                            preT_ps = pagg.tile([128, D], f32, space="PSUM")
                            nc.tensor.transpose(out=preT_ps[:], in_=pre[:],
                                                identity=ident[:])
                            preT = wp.tile([128, D], f32)
                            nc.vector.tensor_copy(out=preT[:], in_=preT_ps[:])
                            agg_ps = pagg.tile([128, hd], f32, space="PSUM")
                            nc.tensor.matmul(agg_ps[:], lhsT=preT[:],
                                             rhs=wv_sb[li][:, h * hd:(h + 1) * hd],
                                             start=True, stop=True)
                            # + denraw * bv_h, then * rec
                            bvt = wp.tile([128, hd], f32)
                            nc.vector.tensor_scalar(
                                out=bvt[:], in0=bv_sb[li][:, h * hd:(h + 1) * hd],
                                scalar1=recs[:, h:h + 1], scalar2=None,
                                op0=mybir.AluOpType.mult)
                            nc.vector.tensor_tensor(
                                out=bvt[:], in0=bvt[:], in1=agg_ps[:],
                                op=mybir.AluOpType.add)
                            nc.vector.tensor_scalar(
                                out=ot[:, h * hd:(h + 1) * hd], in0=bvt[:],
                                scalar1=recs[:, H + h:H + h + 1], scalar2=None,
                                op0=mybir.AluOpType.mult)
                        else:
                            nc.vector.memset(ot[:, h * hd:(h + 1) * hd], 0.0)
                    # + skip, relu
                    nc.vector.tensor_tensor(out=ot[:], in0=ot[:],
                                            in1=qts[:, H * D + H:H * D + H + D],
                                            op=mybir.AluOpType.add)
                    ht = wp.tile([128, D], f32)
                    nc.scalar.activation(ht[:], ot[:],
                                         mybir.ActivationFunctionType.Relu)
                    if li < 2:
                        nc.gpsimd.indirect_dma_start(
                            out=h_bounce[:],
                            out_offset=bass.IndirectOffsetOnAxis(
                                ap=scat_sb[:, t:t + 1], axis=0),
                            in_=ht[:], in_offset=None)
                    else:
                        nc.tensor.matmul(
                            pool_ps[:], lhsT=oneh_sb[:, t * G:(t + 1) * G],
                            rhs=ht[:], start=(t == 0), stop=(t == TILES - 1))
                if li < 2:
                    nc.gpsimd.collective_compute(
                        "AllGather", mybir.AluOpType.bypass,
                        replica_groups=[list(range(NC))],
                        ins=[h_bounce.ap().opt()],
                        outs=[h_full.ap().opt()])
            pout_sb = cp.tile([G, D], f32)
            nc.vector.tensor_copy(out=pout_sb[:], in_=pool_ps[:])
            nc.sync.dma_start(pool_out[:], pout_sb[:])
    nc.compile()
    return nc


def _prep(x, edge_index, batch, weights):
    src = np.asarray(edge_index[0], dtype=np.int64)
    dst = np.asarray(edge_index[1], dtype=np.int64)
    batch = np.asarray(batch, dtype=np.int64)
    deg = np.bincount(dst, minlength=N)

    # incoming edge lists grouped by dst
    order = np.argsort(dst, kind="stable")
    src_sorted = src[order]
    starts = np.concatenate([[0], np.cumsum(deg)]).astype(np.int64)

    perm = np.zeros((NC, PADLOC), dtype=np.int64)
    degs = np.zeros((NC, PADLOC), dtype=np.int64)
    valid = np.zeros((NC, PADLOC), dtype=bool)
    for c in range(NC):
        ids = np.arange(c * NLOC, (c + 1) * NLOC)
        d = deg[ids]
        o = np.argsort(-d, kind="stable")
        perm[c, :NLOC] = ids[o]
        degs[c, :NLOC] = d[o]
        valid[c, :NLOC] = True

    Kt = degs.reshape(NC, TILES, 128).max(axis=(0, 2)).astype(np.int64)
    SLOTS = int(Kt.sum())
    off = np.concatenate([[0], np.cumsum(Kt)]).astype(int)

    def pad_map(ids):
        return (ids // NLOC) * PADLOC + (ids % NLOC)

    ins = []
    for c in range(NC):
        gnat = np.zeros((128, SLOTS), dtype=np.int32)
        mb = np.full((128, SLOTS), -1e30, dtype=np.float32)
        for t in range(TILES):
            K = int(Kt[t])
            for p in range(128):
                i = t * 128 + p
                if not valid[c, i]:
                    continue
                n_id = perm[c, i]
                d = int(deg[n_id])
                if d > 0:
                    ss = src_sorted[starts[n_id]:starts[n_id + 1]]
                    gnat[p, off[t]:off[t] + d] = ss
                    mb[p, off[t]:off[t] + d] = 0.0
        gpad = (pad_map(gnat.astype(np.int64))).astype(np.int32)
        pn = np.zeros((128, TILES), dtype=np.int32)
        ppd = np.zeros((128, TILES), dtype=np.int32)
        sc = np.zeros((128, TILES), dtype=np.int32)
        ohb = np.zeros((128, TILES * G), dtype=np.float32)
        ndum = 0
        for t in range(TILES):
            for p in range(128):
                i = t * 128 + p
                if valid[c, i]:
                    n_id = perm[c, i]
                    pn[p, t] = n_id
                    ppd[p, t] = pad_map(np.int64(n_id))
                    sc[p, t] = n_id - c * NLOC
                    ohb[p, t * G + int(batch[n_id])] = 1.0
                else:
                    pn[p, t] = 0
                    ppd[p, t] = 0
                    sc[p, t] = NLOC + ndum
                    ndum += 1
        m = dict(xin=np.ascontiguousarray(x, dtype=np.float32),
                 pnat=pn, ppad=ppd, scat=sc, gnat=gnat, gpad=gpad,
                 mbig=mb, oneh=ohb)
        ins.append(m)

    # folded weights
    for li in range(3):
        H = LAYER_HEADS[li]
        hd = LAYER_HD[li]
        Wq, bq, Wk, bk, Wv, bv, Ws, bs = weights[li]
        din = Wq.shape[0]
        s = 1.0 / np.sqrt(hd)
        Wq_h = Wq.reshape(din, H, hd)
        Wk_h = Wk.reshape(din, H, hd)
        bq_h = bq.reshape(H, hd)
        bk_h = bk.reshape(H, hd)
        wt = np.zeros((128, H * 128 + H + 128), dtype=np.float32)
        bt = np.zeros((128, H * 128 + H + 128), dtype=np.float32)
        for h in range(H):
            wt[:, h * 128:(h + 1) * 128] = (Wq_h[:, h] @ Wk_h[:, h].T) * s
            bt[:, h * 128:(h + 1) * 128] = ((bq_h[h] @ Wk_h[:, h].T) * s)[None, :]
            wt[:, H * 128 + h] = (Wq_h[:, h] @ bk_h[h]) * s
            bt[:, H * 128 + h] = float(bq_h[h] @ bk_h[h]) * s
        wt[:, H * 128 + H:] = Ws
        bt[:, H * 128 + H:] = bs[None, :]
        wv = np.zeros((128, 128), dtype=np.float32)
        bvb = np.zeros((128, 128), dtype=np.float32)
        Wv_h = Wv.reshape(din, H, hd)
        bv_h = bv.reshape(H, hd)
        for h in range(H):
            wv[:, h * hd:(h + 1) * hd] = Wv_h[:, h]
            bvb[:, h * hd:(h + 1) * hd] = bv_h[h][None, :]
        for m in ins:
            m[f"wt{li}"] = wt
            m[f"bt{li}"] = bt
            m[f"wv{li}"] = wv
            m[f"bv{li}"] = bvb
    return ins, Kt, batch


def kernel(**inputs):
    x = np.asarray(inputs["x"], dtype=np.float32)
    weights = []
    for li in range(1, 4):
        weights.append(tuple(np.asarray(inputs[f"{nm}{li}"], dtype=np.float32)
                             for nm in ("Wq", "bq", "Wk", "bk", "Wv", "bv", "Ws", "bs")))
    ins, Kt, batch = _prep(x, inputs["edge_index"], inputs["batch"], weights)

    key = tuple(Kt.tolist())
    if key not in _CACHE:
        _CACHE[key] = _build({"Kt": Kt})
    nc = _CACHE[key]

    from concourse.bass_utils import run_bass_kernel_spmd
    r = run_bass_kernel_spmd(nc, ins, core_ids=list(range(NC)))
    parts = np.stack([r.results[c]["pool_out"] for c in range(NC)])  # [NC,G,D]
    sums = parts.sum(axis=0)
    cnts = np.bincount(np.asarray(batch, dtype=np.int64), minlength=G).astype(np.float32)
    return (sums / np.maximum(cnts, 1.0)[:, None]).astype(np.float32)


# revision 8
# speedup vs baseline: 36.3973x; 4.8362x over previous
"""TRN2 Bass kernel for nn_GraphVectorEncoder (3-layer TransformerConv GNN + mean pool).

Strategy: partition nodes (and incoming edges) across 8 cores by contiguous
dst ranges. Per core, nodes are degree-sorted into 128-row tiles with K edge
slots along the free axis (dense-bucket layout), so segment softmax = row-wise
ops. Algebraic folding removes k/v tables entirely:
    alpha = x_src . (Wk_h @ q_dst)/sqrt(hd) + c_dst
    agg   = (sum_j p_j x_src_j) @ Wv_h + (sum p) bv_h
Edge phase gathers only x_src rows (512B) via indirect DMA. Layer boundaries
all-gather h across cores. Mean-pool partials are returned per core and
combined on host.
"""

import numpy as np

N, E, G = 50000, 800000, 64
D = 128
NC = 8
NLOC = N // NC           # 6250
TILES = (NLOC + 127) // 128   # 49
PADLOC = TILES * 128     # 6272
NPAD = NC * PADLOC       # 50176
LAYER_HEADS = [2, 2, 1]
LAYER_HD = [64, 64, 128]

_CACHE = {}


def _build(meta):
    import concourse.bass as bass
    import concourse.mybir as mybir
    import concourse.tile as tile
    from concourse import bacc
    from concourse.masks import make_identity

    Kt = meta["Kt"]
    SLOTS = int(Kt.sum())
    off = np.concatenate([[0], np.cumsum(Kt)]).astype(int)

    nc = bacc.Bacc("TRN2", target_bir_lowering=False, debug=False, num_devices=NC)
    f32 = mybir.dt.float32
    i32 = mybir.dt.int32

    xin = nc.dram_tensor("xin", [N, D], f32, kind="ExternalInput")
    pnat = nc.dram_tensor("pnat", [128, TILES], i32, kind="ExternalInput")
    ppad = nc.dram_tensor("ppad", [128, TILES], i32, kind="ExternalInput")
    scat = nc.dram_tensor("scat", [128, TILES], i32, kind="ExternalInput")
    gnat = nc.dram_tensor("gnat", [128, SLOTS], i32, kind="ExternalInput")
    gpad = nc.dram_tensor("gpad", [128, SLOTS], i32, kind="ExternalInput")
    mbig = nc.dram_tensor("mbig", [128, SLOTS], f32, kind="ExternalInput")
    oneh = nc.dram_tensor("oneh", [128, TILES * G], f32, kind="ExternalInput")
    Fs = [2 * 128 + 2 + 128, 2 * 128 + 2 + 128, 128 + 1 + 128]
    wts, bts, wvs, bvs = [], [], [], []
    for li in range(3):
        wts.append(nc.dram_tensor(f"wt{li}", [128, Fs[li]], f32, kind="ExternalInput"))
        bts.append(nc.dram_tensor(f"bt{li}", [128, Fs[li]], f32, kind="ExternalInput"))
        wvs.append(nc.dram_tensor(f"wv{li}", [128, 128], f32, kind="ExternalInput"))
        bvs.append(nc.dram_tensor(f"bv{li}", [128, 128], f32, kind="ExternalInput"))
    pool_out = nc.dram_tensor("pool_out", [G, D], f32, kind="ExternalOutput")

    h_bounce = nc.dram_tensor("h_bounce", [PADLOC, D], f32)
    h_full = nc.dram_tensor("h_full", [NPAD, D], f32)

    with tile.TileContext(nc) as tc:
        with (
            tc.tile_pool(name="const", bufs=1) as cp,
            tc.tile_pool(name="work", bufs=2) as wp,
            tc.tile_pool(name="gw", bufs=3) as gp,
            tc.tile_pool(name="scr", bufs=1) as sp,
            tc.tile_pool(name="ps", bufs=2, space="PSUM") as pp,
            tc.tile_pool(name="ps_agg", bufs=1, space="PSUM") as pagg,
            tc.tile_pool(name="pool_ps", bufs=1, space="PSUM") as ppool,
        ):
            # ---- constants ----
            ident = cp.tile([128, 128], f32)
            make_identity(nc, ident[:])
            pnat_sb = cp.tile([128, TILES], i32)
            nc.sync.dma_start(pnat_sb[:], pnat[:])
            ppad_sb = cp.tile([128, TILES], i32)
            nc.sync.dma_start(ppad_sb[:], ppad[:])
            scat_sb = cp.tile([128, TILES], i32)
            nc.sync.dma_start(scat_sb[:], scat[:])
            gnat_sb = cp.tile([128, SLOTS], i32)
            nc.sync.dma_start(gnat_sb[:], gnat[:])
            gpad_sb = cp.tile([128, SLOTS], i32)
            nc.sync.dma_start(gpad_sb[:], gpad[:])
            mbig_sb = cp.tile([128, SLOTS], f32)
            nc.sync.dma_start(mbig_sb[:], mbig[:])
            oneh_sb = cp.tile([128, TILES * G], f32)
            nc.sync.dma_start(oneh_sb[:], oneh[:])
            wt_sb, bt_sb, wv_sb, bv_sb = [], [], [], []
            for li in range(3):
                w = cp.tile([128, Fs[li]], f32)
                nc.sync.dma_start(w[:], wts[li][:])
                wt_sb.append(w)
                b = cp.tile([128, Fs[li]], f32)
                nc.sync.dma_start(b[:], bts[li][:])
                bt_sb.append(b)
                wv = cp.tile([128, 128], f32)
                nc.sync.dma_start(wv[:], wvs[li][:])
                wv_sb.append(wv)
                bv = cp.tile([128, 128], f32)
                nc.sync.dma_start(bv[:], bvs[li][:])
                bv_sb.append(bv)

            pool_ps = ppool.tile([G, D], f32, space="PSUM")

            for li in range(3):
                H = LAYER_HEADS[li]
                hd = LAYER_HD[li]
                F = Fs[li]
                src_tbl = xin if li == 0 else h_full
                perm_sb = pnat_sb if li == 0 else ppad_sb
                gidx_sb = gnat_sb if li == 0 else gpad_sb

                for t in range(TILES):
                    K = int(Kt[t])
                    o0 = int(off[t])
                    # node features for this tile
                    xg = wp.tile([128, D], f32)
                    nc.gpsimd.indirect_dma_start(
                        out=xg[:], out_offset=None, in_=src_tbl[:],
                        in_offset=bass.IndirectOffsetOnAxis(
                            ap=perm_sb[:, t:t + 1], axis=0))
                    xgT_ps = pp.tile([128, D], f32, space="PSUM")
                    nc.tensor.transpose(out=xgT_ps[:], in_=xg[:], identity=ident[:])
                    xT = wp.tile([128, D], f32)
                    nc.vector.tensor_copy(out=xT[:], in_=xgT_ps[:])
                    p1 = pp.tile([128, F], f32, space="PSUM")
                    nc.tensor.matmul(p1[:], lhsT=xT[:], rhs=wt_sb[li][:],
                                     start=True, stop=True)
                    qts = wp.tile([128, F], f32)
                    nc.vector.tensor_tensor(out=qts[:], in0=p1[:], in1=bt_sb[li][:],
                                            op=mybir.AluOpType.add)

                    if K > 0:
                        Gt = gp.tile([128, K * D], f32)
                        for j in range(K):
                            nc.gpsimd.indirect_dma_start(
                                out=Gt[:, j * D:(j + 1) * D], out_offset=None,
                                in_=src_tbl[:],
                                in_offset=bass.IndirectOffsetOnAxis(
                                    ap=gidx_sb[:, o0 + j:o0 + j + 1], axis=0))
                        w_sc = sp.tile([128, K * D], f32)
                        alpha = wp.tile([128, H * K], f32)
                        pexp = wp.tile([128, H * K], f32)
                        recs = wp.tile([128, 2 * H], f32)  # [denraw | rec] per head
                        for h in range(H):
                            qb = bass.AP(qts.tensor, qts[:].offset + h * D,
                                         [qts[:].ap[0], [0, K], [1, D]])
                            nc.vector.tensor_tensor(
                                out=w_sc[:].rearrange("p (k d) -> p k d", k=K),
                                in0=Gt[:].rearrange("p (k d) -> p k d", k=K),
                                in1=qb, op=mybir.AluOpType.mult)
                            nc.vector.tensor_reduce(
                                out=alpha[:, h * K:(h + 1) * K],
                                in_=w_sc[:].rearrange("p (k d) -> p k d", k=K),
                                axis=mybir.AxisListType.X, op=mybir.AluOpType.add)
                            # + c term (per-partition scalar)
                            nc.vector.tensor_scalar(
                                out=alpha[:, h * K:(h + 1) * K],
                                in0=alpha[:, h * K:(h + 1) * K],
                                scalar1=qts[:, H * D + h:H * D + h + 1],
                                scalar2=None, op0=mybir.AluOpType.add)
                            # + (-1e30) on padded slots
                            nc.vector.tensor_tensor(
                                out=alpha[:, h * K:(h + 1) * K],
                                in0=alpha[:, h * K:(h + 1) * K],
                                in1=mbig_sb[:, o0:o0 + K],
                                op=mybir.AluOpType.add)
                            amax = wp.tile([128, 1], f32)
                            nc.vector.tensor_reduce(
                                out=amax[:], in_=alpha[:, h * K:(h + 1) * K],
                                axis=mybir.AxisListType.X, op=mybir.AluOpType.max)
                            nc.vector.tensor_scalar(
                                out=alpha[:, h * K:(h + 1) * K],
                                in0=alpha[:, h * K:(h + 1) * K],
                                scalar1=amax[:, :1], scalar2=None,
                                op0=mybir.AluOpType.subtract)
                            nc.scalar.activation(pexp[:, h * K:(h + 1) * K],
                                                 alpha[:, h * K:(h + 1) * K],
                                                 mybir.ActivationFunctionType.Exp)
                            nc.vector.tensor_reduce(
                                out=recs[:, h:h + 1], in_=pexp[:, h * K:(h + 1) * K],
                                axis=mybir.AxisListType.X, op=mybir.AluOpType.add)
                            den = wp.tile([128, 1], f32)
                            nc.vector.tensor_scalar(
                                out=den[:], in0=recs[:, h:h + 1], scalar1=1e-16,
                                scalar2=None, op0=mybir.AluOpType.add)
                            nc.vector.reciprocal(out=recs[:, H + h:H + h + 1],
                                                 in_=den[:])

                    ot = wp.tile([128, D], f32)  # output features (pre-relu)
                    for h in range(H):
                        if K > 0:
                            pre = wp.tile([128, D], f32)
                            tmp_a = wp.tile([128, D], f32)
                            tmp_b = wp.tile([128, D], f32)
                            tmps = [tmp_a, tmp_b]
                            nc.scalar.activation(
                                pre[:], Gt[:, :D],
                                mybir.ActivationFunctionType.Copy,
                                scale=pexp[:, h * K:h * K + 1])
                            for j in range(1, K):
                                tm = tmps[j % 2]
                                nc.scalar.activation(
                                    tm[:], Gt[:, j * D:(j + 1) * D],
                                    mybir.ActivationFunctionType.Copy,
                                    scale=pexp[:, h * K + j:h * K + j + 1])
                                nc.vector.tensor_tensor(
                                    out=pre[:], in0=pre[:], in1=tm[:],
                                    op=mybir.AluOpType.add)
                            preT_ps = pagg.tile([128, D], f32, space="PSUM")
                            nc.tensor.transpose(out=preT_ps[:], in_=pre[:],
                                                identity=ident[:])
                            preT = wp.tile([128, D], f32)
                            nc.vector.tensor_copy(out=preT[:], in_=preT_ps[:])
                            agg_ps = pagg.tile([128, hd], f32, space="PSUM")
                            nc.tensor.matmul(agg_ps[:], lhsT=preT[:],
                                             rhs=wv_sb[li][:, h * hd:(h + 1) * hd],
                                             start=True, stop=True)
                            # + denraw * bv_h, then * rec
                            bvt = wp.tile([128, hd], f32)
                            nc.vector.tensor_scalar(
                                out=bvt[:], in0=bv_sb[li][:, h * hd:(h + 1) * hd],
                                scalar1=recs[:, h:h + 1], scalar2=None,
                                op0=mybir.AluOpType.mult)
                            nc.vector.tensor_tensor(
                                out=bvt[:], in0=bvt[:], in1=agg_ps[:],
                                op=mybir.AluOpType.add)
                            nc.vector.tensor_scalar(
                                out=ot[:, h * hd:(h + 1) * hd], in0=bvt[:],
                                scalar1=recs[:, H + h:H + h + 1], scalar2=None,
                                op0=mybir.AluOpType.mult)
                        else:
                            nc.vector.memset(ot[:, h * hd:(h + 1) * hd], 0.0)
                    # + skip, relu
                    nc.vector.tensor_tensor(out=ot[:], in0=ot[:],
                                            in1=qts[:, H * D + H:H * D + H + D],
                                            op=mybir.AluOpType.add)
                    ht = wp.tile([128, D], f32)
                    nc.scalar.activation(ht[:], ot[:],
                                         mybir.ActivationFunctionType.Relu)
                    if li < 2:
                        nc.gpsimd.indirect_dma_start(
                            out=h_bounce[:],
                            out_offset=bass.IndirectOffsetOnAxis(
                                ap=scat_sb[:, t:t + 1], axis=0),
                            in_=ht[:], in_offset=None)
                    else:
                        nc.tensor.matmul(
                            pool_ps[:], lhsT=oneh_sb[:, t * G:(t + 1) * G],
                            rhs=ht[:], start=(t == 0), stop=(t == TILES - 1))
                if li < 2:
                    nc.gpsimd.collective_compute(
                        "AllGather", mybir.AluOpType.bypass,
                        replica_groups=[list(range(NC))],
                        ins=[h_bounce.ap().opt()],
                        outs=[h_full.ap().opt()])
            pout_sb = cp.tile([G, D], f32)
            nc.vector.tensor_copy(out=pout_sb[:], in_=pool_ps[:])
            nc.sync.dma_start(pool_out[:], pout_sb[:])
    nc.compile()
    return nc


def _prep(x, edge_index, batch, weights):
    src = np.asarray(edge_index[0], dtype=np.int64)
    dst = np.asarray(edge_index[1], dtype=np.int64)
    batch = np.asarray(batch, dtype=np.int64)
    deg = np.bincount(dst, minlength=N)

    # incoming edge lists grouped by dst
    order = np.argsort(dst, kind="stable")
    src_sorted = src[order]
    starts = np.concatenate([[0], np.cumsum(deg)]).astype(np.int64)

    perm = np.zeros((NC, PADLOC), dtype=np.int64)
    degs = np.zeros((NC, PADLOC), dtype=np.int64)
    valid = np.zeros((NC, PADLOC), dtype=bool)
    for c in range(NC):
        ids = np.arange(c * NLOC, (c + 1) * NLOC)
        d = deg[ids]
        o = np.argsort(-d, kind="stable")
        perm[c, :NLOC] = ids[o]
        degs[c, :NLOC] = d[o]
        valid[c, :NLOC] = True

    Kt = degs.reshape(NC, TILES, 128).max(axis=(0, 2)).astype(np.int64)
    SLOTS = int(Kt.sum())
    off = np.concatenate([[0], np.cumsum(Kt)]).astype(int)

    def pad_map(ids):
        return (ids // NLOC) * PADLOC + (ids % NLOC)

    ins = []
    for c in range(NC):
        gnat = np.zeros((128, SLOTS), dtype=np.int32)
        mb = np.full((128, SLOTS), -1e30, dtype=np.float32)
        for t in range(TILES):
            K = int(Kt[t])
            for p in range(128):
                i = t * 128 + p
                if not valid[c, i]:
                    continue
                n_id = perm[c, i]
                d = int(deg[n_id])
                if d > 0:
                    ss = src_sorted[starts[n_id]:starts[n_id + 1]]
                    gnat[p, off[t]:off[t] + d] = ss
                    mb[p, off[t]:off[t] + d] = 0.0
        gpad = (pad_map(gnat.astype(np.int64))).astype(np.int32)
        pn = np.zeros((128, TILES), dtype=np.int32)
        ppd = np.zeros((128, TILES), dtype=np.int32)
        sc = np.zeros((128, TILES), dtype=np.int32)
        ohb = np.zeros((128, TILES * G), dtype=np.float32)
        ndum = 0
        for t in range(TILES):
            for p in range(128):
                i = t * 128 + p
                if valid[c, i]:
                    n_id = perm[c, i]
                    pn[p, t] = n_id
                    ppd[p, t] = pad_map(np.int64(n_id))
                    sc[p, t] = n_id - c * NLOC
                    ohb[p, t * G + int(batch[n_id])] = 1.0
                else:
                    pn[p, t] = 0
                    ppd[p, t] = 0
                    sc[p, t] = NLOC + ndum
                    ndum += 1
        m = dict(xin=np.ascontiguousarray(x, dtype=np.float32),
                 pnat=pn, ppad=ppd, scat=sc, gnat=gnat, gpad=gpad,
                 mbig=mb, oneh=ohb)
        ins.append(m)

    # folded weights
    for li in range(3):
        H = LAYER_HEADS[li]
        hd = LAYER_HD[li]
        Wq, bq, Wk, bk, Wv, bv, Ws, bs = weights[li]
        din = Wq.shape[0]
        s = 1.0 / np.sqrt(hd)
        Wq_h = Wq.reshape(din, H, hd)
        Wk_h = Wk.reshape(din, H, hd)
        bq_h = bq.reshape(H, hd)
        bk_h = bk.reshape(H, hd)
        wt = np.zeros((128, H * 128 + H + 128), dtype=np.float32)
        bt = np.zeros((128, H * 128 + H + 128), dtype=np.float32)
        for h in range(H):
            wt[:, h * 128:(h + 1) * 128] = (Wq_h[:, h] @ Wk_h[:, h].T) * s
            bt[:, h * 128:(h + 1) * 128] = ((bq_h[h] @ Wk_h[:, h].T) * s)[None, :]
            wt[:, H * 128 + h] = (Wq_h[:, h] @ bk_h[h]) * s
            bt[:, H * 128 + h] = float(bq_h[h] @ bk_h[h]) * s
        wt[:, H * 128 + H:] = Ws
        bt[:, H * 128 + H:] = bs[None, :]
        wv = np.zeros((128, 128), dtype=np.float32)
        bvb = np.zeros((128, 128), dtype=np.float32)
        Wv_h = Wv.reshape(din, H, hd)
        bv_h = bv.reshape(H, hd)
        for h in range(H):
            wv[:, h * hd:(h + 1) * hd] = Wv_h[:, h]
            bvb[:, h * hd:(h + 1) * hd] = bv_h[h][None, :]
        for m in ins:
            m[f"wt{li}"] = wt
            m[f"bt{li}"] = bt
            m[f"wv{li}"] = wv
            m[f"bv{li}"] = bvb
    return ins, Kt, batch


def kernel(**inputs):
    x = np.asarray(inputs["x"], dtype=np.float32)
    weights = []
    for li in range(1, 4):
        weights.append(tuple(np.asarray(inputs[f"{nm}{li}"], dtype=np.float32)
                             for nm in ("Wq", "bq", "Wk", "bk", "Wv", "bv", "Ws", "bs")))
    ins, Kt, batch = _prep(x, inputs["edge_index"], inputs["batch"], weights)

    key = tuple(Kt.tolist())
    if key not in _CACHE:
        _CACHE[key] = _build({"Kt": Kt})
    nc = _CACHE[key]

    from concourse.bass_utils import run_bass_kernel_spmd
    r = run_bass_kernel_spmd(nc, ins, core_ids=list(range(NC)))
    parts = np.stack([r.results[c]["pool_out"] for c in range(NC)])  # [NC,G,D]
    sums = parts.sum(axis=0)
    cnts = np.bincount(np.asarray(batch, dtype=np.int64), minlength=G).astype(np.float32)
    return (sums / np.maximum(cnts, 1.0)[:, None]).astype(np.float32)
